# revision 25
# baseline (speedup 1.0000x reference)
"""Trainium2 Bass kernel for nn_Attention_34351148434119 (8 NeuronCores).

Reference computation (faithful quirks included):
  q_proj = hid @ Wq; q, gate = split(q_proj)     # q is DEAD code downstream
  k = hid @ Wk; v = hid @ Wv                     # [B,KV,S,D]
  v = RoPE(v)  (k is NOT roped; q roped but unused)
  scores = (k @ v^T) * sqrt(D) + mask; attn = softmax_t(scores)   # per kv head
  out = (tile_G(attn @ v) * sigmoid(gate)) @ Wo

Sharding: core = b*4 + j  (b = batch, j = rank in 4-core batch group).
Per batch, S=2048 is split into 16 blocks of 128 rows; core j owns blocks
{j, 4+j, 8+j, 12+j} (slot k block = 4k+j) so every core has an identical
causal workload (uniform SPMD graph; per-core specialization only via
staged data).

Pipeline (v1 lessons: collectives starve host DMA queues while in
flight, and the PE clock ramps with uninterrupted streak length):
  1. v-projection only (fp32r: hw-internal bf16 hi/lo split at full PE
     speed), RoPE, stage, AllGather issued EARLY (~t=45us).
  2. k-projection (fp32r) from the resident hid tiles.
  3. Gate matmuls run bf16 from HOST-staged bf16 weights, fully
     preloaded into SBUF before the AG starts - zero DMA during the AG.
  4. Attention per kv head: fp32r scores (logits sigma~105: softmax is
     near-argmax, bf16 anywhere in k/v->scores flips rows), two-phase
     softmax, PE transposes, bf16 attn@v.  Row-major v is derived from
     the gathered d-major v by on-chip transposes (no 2nd AllGather).
  5. Gating + bf16 out-projection from host-staged bf16 Wo, streamed
     through a deep slab ring (no conversion ops).
"""
import sys
import numpy as np
import ml_dtypes

sys.path.insert(0, "/opt/trn_rl_repo")

B, S, HS = 2, 2048, 2048
H, KV, D = 16, 4, 128
G = H // KV
SCALING = float(D) ** 0.5
P = 128
NB = S // P            # 16 row blocks per batch
NCORES = 8
RANKS = 4              # cores per batch group
SLOTS = 4              # owned 128-row blocks per core
ROWS = SLOTS * P       # 512 rows per core
CHUNK = 512            # t-chunk = 4 t-tiles
NCHUNK = S // CHUNK    # 4
KT = HS // P           # 16 contraction tiles
KVD = KV * D
NEG_THRESH = -1e8

_CACHE = {}


def _mask_classes(mask):
    """Classify each (s-slot k, t-chunk c) 512x512 region of the SxS mask.

    0 = skip (everything <= NEG_THRESH: contributes exact 0 after softmax)
    1 = plain (all zeros: no add needed)
    2 = add  (mixed: stage values and add on-chip)
    Slot k rows across all cores = blocks 4k..4k+3 = rows [512k, 512k+512).
    """
    cls = [[0] * NCHUNK for _ in range(SLOTS)]
    for k in range(SLOTS):
        for c in range(NCHUNK):
            reg = mask[512 * k:512 * (k + 1), 512 * c:512 * (c + 1)]
            if (reg <= NEG_THRESH).all():
                cls[k][c] = 0
            elif (reg == 0).all():
                cls[k][c] = 1
            else:
                cls[k][c] = 2
    ok = True
    for k in range(SLOTS):
        comp = [c for c in range(NCHUNK) if cls[k][c] != 0]
        # computed chunks must be a prefix starting at 0
        if comp != list(range(len(comp))) or 0 not in comp:
            ok = False
    if ok:
        # {k : chunk c computed} must be a suffix of slots for each c
        for c in range(NCHUNK):
            ks = [k for k in range(SLOTS) if cls[k][c] != 0]
            if ks != list(range(SLOTS - len(ks), SLOTS)):
                ok = False
    if not ok:
        # fully dense fallback: always correct for any mask
        cls = [[2] * NCHUNK for _ in range(SLOTS)]
    return cls


def _mask_strips(mask, classes, j):
    """Per-core class-2 strips, in (k, c) scan order."""
    strips = []
    for k in range(SLOTS):
        for c in range(NCHUNK):
            if classes[k][c] == 2:
                bi = RANKS * k + j
                strips.append(np.ascontiguousarray(
                    mask[bi * P:(bi + 1) * P, c * CHUNK:(c + 1) * CHUNK]))
    return strips


def _dedup_map(mask, classes):
    """Map each class-2 (k,c) to a unique-strip index, valid for EVERY
    core (cores hold different rows, so strip equality must hold on all
    of them).  Returns (uniq_of_addidx, n_uniq) or None if coreswise
    inconsistent."""
    n_add = sum(1 for k in range(SLOTS) for c in range(NCHUNK)
                if classes[k][c] == 2)
    per_core = []
    for j in range(RANKS):
        strips = _mask_strips(mask, classes, j)
        uniq = []
        idx = []
        for s in strips:
            for ui, u in enumerate(uniq):
                if np.array_equal(s, u):
                    idx.append(ui)
                    break
            else:
                uniq.append(s)
                idx.append(len(uniq) - 1)
        per_core.append(tuple(idx))
    if len(set(per_core)) != 1:
        return tuple(range(n_add)), n_add     # no dedup
    return per_core[0], max(per_core[0]) + 1 if per_core[0] else 0


def _build(classes, uniq_idx, n_uniq):
    from contextlib import ExitStack

    from concourse import bacc, mybir, tile
    from concourse.masks import make_identity

    f32 = mybir.dt.float32
    f32r = mybir.dt.float32r
    bf16 = mybir.dt.bfloat16
    Alu = mybir.AluOpType
    Act = mybir.ActivationFunctionType

    computed = [[c for c in range(NCHUNK) if classes[k][c] != 0] for k in range(SLOTS)]
    add_idx = {}
    for k in range(SLOTS):
        for c in range(NCHUNK):
            if classes[k][c] == 2:
                add_idx[(k, c)] = len(add_idx)
    n_mask = max(n_uniq, 1)
    resident_mask = n_uniq <= 4

    nc = bacc.Bacc("TRN2", target_bir_lowering=False, debug=False,
                   num_devices=NCORES)

    hidT_d = nc.declare_dram_parameter("hidT", [HS, ROWS], f32r, isOutput=False)
    wqg_d = nc.declare_dram_parameter("wqg", [HS, HS], bf16, isOutput=False)
    wk_d = nc.declare_dram_parameter("wk", [HS, KVD], f32r, isOutput=False)
    wv_d = nc.declare_dram_parameter("wv", [HS, KVD], f32r, isOutput=False)
    wo_d = nc.declare_dram_parameter("wo", [HS, HS], bf16, isOutput=False)
    cosT_d = nc.declare_dram_parameter("cosT", [D, ROWS], f32, isOutput=False)
    sinT_d = nc.declare_dram_parameter("sinT", [D, ROWS], f32, isOutput=False)
    mask_d = nc.declare_dram_parameter("maskst", [n_mask, P, CHUNK], f32,
                                       isOutput=False)
    out_d = nc.declare_dram_parameter("out", [ROWS, HS], f32, isOutput=True)

    rg = [[0, 1, 2, 3], [4, 5, 6, 7]]
    NSLAB = 64    # bf16 weight-slab ring (full gate preload, wo streams through)

    with tile.TileContext(nc) as tc, ExitStack() as ctx:
        sb = ctx.enter_context(tc.tile_pool(name="sb", bufs=2))
        ps = ctx.enter_context(tc.tile_pool(name="ps", bufs=8, space="PSUM"))
        dram = ctx.enter_context(tc.tile_pool(name="dram", bufs=1, space="DRAM"))

        # ---- constants ----
        id_f32 = sb.tile([P, P], f32, tag="c_idf")
        id_bf = sb.tile([P, P], bf16, tag="c_idb")
        make_identity(nc, id_f32[:])
        make_identity(nc, id_bf[:])

        # ---- combined k+v projection (fp32r): the front half is at the
        # aggregate HBM roofline, so k matmuls ride the same DMA-bound
        # window as v instead of consuming PE time afterwards ----
        pv = [ps.tile([P, ROWS], f32, tag="ps", name=f"pv{g}") for g in range(KV)]
        pk = [ps.tile([P, ROWS], f32, tag="ps", name=f"pk{g}") for g in range(KV)]
        hidb = []
        hfw = [sb.tile([P, 4 * ROWS], f32r, tag="f32big", bufs=2, name=f"hfw{q}")
               for q in range(4)]
        hid = [hfw[kk // 4][:, (kk % 4) * ROWS:(kk % 4 + 1) * ROWS]
               for kk in range(KT)]
        for kk in range(KT):
            nc.sync.dma_start(hid[kk], hidT_d[kk * P:(kk + 1) * P, :])
            wt = sb.tile([P, 2 * KVD], f32r, tag="wkv", bufs=2, name=f"wv{kk}")
            nc.gpsimd.dma_start(wt[:, :KVD], wv_d[kk * P:(kk + 1) * P, :])
            nc.gpsimd.dma_start(wt[:, KVD:], wk_d[kk * P:(kk + 1) * P, :])
            for g in range(KV):
                nc.tensor.matmul(pv[g][:], wt[:, g * P:(g + 1) * P], hid[kk],
                                 start=(kk == 0), stop=(kk == KT - 1))
            for g in range(KV):
                nc.tensor.matmul(pk[g][:], wt[:, KVD + g * P:KVD + (g + 1) * P],
                                 hid[kk], start=(kk == 0), stop=(kk == KT - 1))
            hb = sb.tile([P, ROWS], bf16, tag="hidb", bufs=16, name=f"hb{kk}")
            nc.scalar.copy(hb[:], hid[kk].bitcast(f32))
            hidb.append(hb)

        # ---- cos/sin/mask after the kv stream (needed only ~t=75) ----
        cosT = sb.tile([D, ROWS], f32, tag="c_cos")
        sinT = sb.tile([D, ROWS], f32, tag="c_sin")
        nc.sync.dma_start(cosT[:], cosT_d[:, :])
        nc.sync.dma_start(sinT[:], sinT_d[:, :])
        mtiles = []
        if resident_mask:
            for u in range(n_uniq):
                mt = sb.tile([P, CHUNK], f32, tag="msk", bufs=max(n_uniq, 1),
                             name=f"mt{u}")
                nc.sync.dma_start(mt[:], mask_d[u, :, :])
                mtiles.append(mt)

        # ---- gate weights: wide slabs on the SCALAR dma queue ----
        wqb = []
        for kk in range(KT):
            t = sb.tile([P, HS], bf16, tag="wslab", bufs=16, name=f"wq{kk}")
            nc.scalar.dma_start(t[:], wqg_d[kk * P:(kk + 1) * P, :])
            wqb.append(t)

        # ---- RoPE v pre-AG; per-g AllGather (4 small collectives) so
        # head 0's data lands well before attention needs it ----
        vt_in = dram.tile([KVD, ROWS], f32)
        vt_all = [dram.tile([RANKS * P, ROWS], f32, name=f"vtall{g}")
                  for g in range(KV)]
        for g in range(KV):
            vr = sb.tile([P, ROWS], f32, tag="vraw", bufs=2, name=f"vr{g}")
            nc.scalar.copy(vr[:], pv[g][:])
            rot = sb.tile([P, ROWS], f32, tag="vrot", bufs=2, name=f"rot{g}")
            nc.vector.tensor_scalar_mul(rot[0:64, :], vr[64:128, :], -1.0)
            nc.vector.tensor_copy(rot[64:128, :], vr[0:64, :])
            nc.vector.tensor_mul(vr[:], vr[:], cosT[:])
            nc.vector.tensor_mul(rot[:], rot[:], sinT[:])
            nc.vector.tensor_add(vr[:], vr[:], rot[:])
            nc.gpsimd.dma_start(vt_in[g * P:(g + 1) * P, :], vr[:])
            nc.gpsimd.collective_compute(
                "AllGather", mybir.AluOpType.bypass, replica_groups=rg,
                ins=[vt_in[g * P:(g + 1) * P, :].opt()],
                outs=[vt_all[g].opt()])

        kT = []   # per g: [128 d, 512 rows] f32r, pre-scaled by sqrt(D)
        for g in range(KV):
            t = sb.tile([P, ROWS], f32r, tag="kT", bufs=4, name=f"kT{g}")
            nc.scalar.mul(t[:], pk[g][:], SCALING)
            kT.append(t)

        # ---- gathered v: per-g wide tiles recycling the hid ring
        # (bufs=2).  g=0,1 load right after the AG; g=2,3 are deferred
        # into the attention pipeline so their ring waits never block
        # the queue mid-phase. ----
        vtcg = []

        def emit_vtc_load(g):
            t = vtcg[g]
            for c in range(NCHUNK):
                for r in range(RANKS):
                    nc.sync.dma_start(
                        t[:, (4 * c + r) * P:(4 * c + r + 1) * P],
                        vt_all[g][r * P:(r + 1) * P,
                                  c * P:(c + 1) * P].bitcast(f32r))

        for g in range(KV):
            vtcg.append(sb.tile([P, 4 * CHUNK], f32r, tag="f32big", bufs=2,
                                name=f"vtcg{g}"))
        emit_vtc_load(0)
        emit_vtc_load(1)

        # ---- gate matmul (bf16) + fused sigmoid: zero DMA during AG ----
        sigT = [None] * 16
        for nblk in range(4):
            for m in range(4):
                pg = ps.tile([P, ROWS], f32, tag="ps", name=f"pg{nblk}_{m}")
                for kk in range(KT):
                    nc.tensor.matmul(
                        pg[:], wqb[kk][:, nblk * CHUNK + m * P:nblk * CHUNK + (m + 1) * P],
                        hidb[kk][:], start=(kk == 0), stop=(kk == KT - 1))
                t = sb.tile([P, ROWS], bf16, tag="sigT", bufs=16, name=f"sig{nblk}_{m}")
                nc.scalar.activation(t[:], pg[:], Act.Sigmoid)
                sigT[nblk * 4 + m] = t

        # ---- attention: flat software pipeline over (g, slot) tasks.
        # scores of slot k+1/k+2 and transposes of slot k-1 overlap the
        # Act/DVE softmax chain of slot k, so the PE never drains. ----
        avT = [None] * KV
        vrows = {}
        attnTs = {}
        sstate = {}
        tstate = {}
        pavs = {}
        def emit_vrow(g):
            vl = []
            for bi in range(NB):
                tp = ps.tile([P, P], f32, tag="ps", name=f"tvp{g}_{bi}")
                nc.tensor.transpose(
                    tp[:], vtcg[g][:, bi * P:(bi + 1) * P].bitcast(f32), id_f32[:])
                t = sb.tile([P, P], bf16, tag="vrow", bufs=16,
                            name=f"vrow{g}_{bi}")
                if bi % 2:
                    nc.scalar.copy(t[:], tp[:])
                else:
                    nc.vector.tensor_copy(t[:], tp[:])
                vl.append(t)
            vrows[g] = vl
            attnTs[g] = [sb.tile([P, ROWS], bf16, tag="attnT", bufs=16,
                                 name=f"attnT{g}_{bi}") for bi in range(NB)]

        def emit_scores(g, k):
            comp = computed[k]
            pscs = []
            cms = []
            for ci, c in enumerate(comp):
                psc = ps.tile([P, CHUNK], f32, tag="ps", name=f"psc{g}_{k}_{ci}")
                nc.tensor.matmul(psc[:], kT[g][:, k * P:(k + 1) * P],
                                 vtcg[g][:, c * CHUNK:(c + 1) * CHUNK],
                                 start=True, stop=True)
                if classes[k][c] == 2:
                    ai = add_idx[(k, c)]
                    if resident_mask:
                        mt = mtiles[uniq_idx[ai]]
                    else:
                        mt = sb.tile([P, CHUNK], f32, tag="msk", bufs=4,
                                     name=f"msk{g}_{k}_{c}")
                        nc.gpsimd.dma_start(mt[:], mask_d[ai, :, :])
                    nc.vector.tensor_add(psc[:], psc[:], mt[:])
                cm = sb.tile([P, 1], f32, tag="stat", bufs=32,
                             name=f"cm{g}_{k}_{ci}")
                nc.vector.tensor_reduce(cm[:], psc[:], mybir.AxisListType.X,
                                        Alu.max, negate=True)
                pscs.append(psc)
                cms.append(cm)
            sstate[(g, k)] = (pscs, cms, comp)

        def emit_softmax(g, k):
            pscs, cms, comp = sstate.pop((g, k))
            nchk = len(comp)
            attn = sb.tile([P, CHUNK * nchk], bf16, tag="attn", bufs=2,
                           padded_shape=[P, CHUNK * NCHUNK],
                           name=f"attn{g}_{k}")
            mneg = cms[0]   # -max
            for ci in range(1, nchk):
                mnew = sb.tile([P, 1], f32, tag="stat", bufs=32,
                               name=f"mn{g}_{k}_{ci}")
                nc.vector.tensor_tensor(mnew[:], mneg[:], cms[ci][:], Alu.min)
                mneg = mnew
            tot = None
            for ci in range(nchk):
                csum = sb.tile([P, 1], f32, tag="stat", bufs=32,
                               name=f"cs{g}_{k}_{ci}")
                nc.scalar.activation(attn[:, ci * CHUNK:(ci + 1) * CHUNK],
                                     pscs[ci][:], Act.Exp, bias=mneg[:],
                                     accum_out=csum[:])
                if tot is None:
                    tot = csum
                else:
                    t2 = sb.tile([P, 1], f32, tag="stat", bufs=32,
                                 name=f"tt{g}_{k}_{ci}")
                    nc.vector.tensor_add(t2[:], tot[:], csum[:])
                    tot = t2
            rinv = sb.tile([P, 1], f32, tag="stat", bufs=32,
                           name=f"ri{g}_{k}")
            nc.vector.reciprocal(rinv[:], tot[:])
            for ci in range(nchk):
                nc.vector.tensor_scalar_mul(
                    attn[:, ci * CHUNK:(ci + 1) * CHUNK],
                    attn[:, ci * CHUNK:(ci + 1) * CHUNK], rinv[:])
            tstate[(g, k)] = (attn, comp)

        def emit_transposes(g, k):
            attn, comp = tstate.pop((g, k))
            attnT = attnTs[g]
            for ci, c in enumerate(comp):
                for i in range(4):
                    bi = 4 * c + i
                    tp = ps.tile([P, P], bf16, tag="ps", name=f"tap{g}_{k}_{bi}")
                    nc.tensor.transpose(
                        tp[:], attn[:, ci * CHUNK + i * P:ci * CHUNK + (i + 1) * P],
                        id_bf[:])
                    if i % 2:
                        nc.scalar.copy(attnT[bi][:, k * P:(k + 1) * P], tp[:])
                    else:
                        nc.vector.tensor_copy(attnT[bi][:, k * P:(k + 1) * P], tp[:])

        def emit_av_mm(g):
            pav = ps.tile([P, ROWS], f32, tag="ps", name=f"pav{g}")
            first = True
            for bi in range(NB):
                ks = [k for k in range(SLOTS) if (bi // RANKS) in computed[k]]
                if not ks:
                    continue
                kmin = ks[0]
                nc.tensor.matmul(pav[:, kmin * P:ROWS], vrows[g][bi][:],
                                 attnTs[g][bi][:, kmin * P:ROWS],
                                 start=first, stop=(bi == NB - 1))
                first = False
            pavs[g] = pav

        def emit_av_drain(g):
            t = sb.tile([P, ROWS], bf16, tag="avT", bufs=4, name=f"avT{g}")
            nc.vector.tensor_copy(t[:], pavs.pop(g)[:])
            avT[g] = t
            for i in range(G):
                s = sigT[4 * g + i]
                nc.vector.tensor_mul(s[:], t[:], s[:])

        emit_vrow(0)
        emit_scores(0, 0)
        emit_scores(0, 1)
        for g in range(KV):
            emit_softmax(g, 0)
            emit_scores(g, 2)
            emit_softmax(g, 1)
            emit_transposes(g, 0)
            emit_scores(g, 3)
            emit_softmax(g, 2)
            emit_transposes(g, 1)
            emit_softmax(g, 3)
            emit_transposes(g, 2)
            emit_transposes(g, 3)
            emit_av_mm(g)
            if g + 2 < KV:
                emit_vtc_load(g + 2)
            if g + 1 < KV:
                emit_vrow(g + 1)
                emit_scores(g + 1, 0)
                emit_scores(g + 1, 1)
            emit_av_drain(g)

        gat = sigT   # gating applied in-place per-g inside the pipeline

        # ---- out projection (bf16, host-staged wide slabs, ring reuse) ----
        wob = []
        for cc in range(KT):
            t = sb.tile([P, HS], bf16, tag="wslab", bufs=16, name=f"wo{cc}")
            nc.sync.dma_start(t[:], wo_d[cc * P:(cc + 1) * P, :])
            wob.append(t)
        for nblk in range(4):
            for rt in range(SLOTS):
                po = ps.tile([P, CHUNK], f32, tag="ps")
                for cc in range(KT):
                    nc.tensor.matmul(
                        po[:], gat[cc][:, rt * P:(rt + 1) * P],
                        wob[cc][:, nblk * CHUNK:(nblk + 1) * CHUNK],
                        start=(cc == 0), stop=(cc == KT - 1))
                t = sb.tile([P, CHUNK], f32, tag="oev", bufs=2)
                nc.scalar.copy(t[:], po[:])
                nc.sync.dma_start(
                    out_d[rt * P:(rt + 1) * P, nblk * CHUNK:(nblk + 1) * CHUNK], t[:])

    nc.compile()
    return nc


def kernel(hidden_states, cos, sin, attention_mask, Wq, Wk, Wv, Wo):
    from concourse.bass_utils import run_bass_kernel_spmd

    hidden_states = np.asarray(hidden_states, dtype=np.float32)
    cos = np.asarray(cos, dtype=np.float32)
    sin = np.asarray(sin, dtype=np.float32)
    mask = np.asarray(attention_mask, dtype=np.float32)[0, 0]
    Wq = np.asarray(Wq, dtype=np.float32)
    Wk = np.asarray(Wk, dtype=np.float32)
    Wv = np.asarray(Wv, dtype=np.float32)
    Wo = np.asarray(Wo, dtype=np.float32)

    classes = _mask_classes(mask)
    uniq_idx, n_uniq = _dedup_map(mask, classes)
    key = (tuple(tuple(r) for r in classes), tuple(uniq_idx), n_uniq)
    if key not in _CACHE:
        _CACHE[key] = _build(classes, uniq_idx, n_uniq)
    nc = _CACHE[key]

    wqg = np.ascontiguousarray(Wq[:, HS:]).astype(ml_dtypes.bfloat16)
    wob = Wo.astype(ml_dtypes.bfloat16)

    in_maps = []
    for core in range(NCORES):
        b, j = divmod(core, RANKS)
        blocks = [RANKS * k + j for k in range(SLOTS)]
        rows = np.concatenate([np.arange(bi * P, (bi + 1) * P) for bi in blocks])
        strips = _mask_strips(mask, classes, j)
        if n_uniq > 0 and len(set(uniq_idx)) != len(strips):
            # staged per unique index
            uniq_strips = [None] * (max(uniq_idx) + 1)
            for si, ui in enumerate(uniq_idx):
                if uniq_strips[ui] is None:
                    uniq_strips[ui] = strips[si]
            strips = uniq_strips
        if not strips:
            strips = [np.zeros((P, CHUNK), np.float32)]
        hidT = np.ascontiguousarray(hidden_states[b][rows].T)
        in_maps.append({
            "hidT": hidT,
            "wqg": wqg,
            "wk": Wk,
            "wv": Wv,
            "wo": wob,
            "cosT": np.ascontiguousarray(cos[b][rows].T),
            "sinT": np.ascontiguousarray(sin[b][rows].T),
            "maskst": np.ascontiguousarray(np.stack(strips)),
        })

    res = run_bass_kernel_spmd(nc, in_maps, core_ids=list(range(NCORES)))

    out = np.empty((B, S, HS), np.float32)
    for core in range(NCORES):
        b, j = divmod(core, RANKS)
        o = res.results[core]["out"]
        for k in range(SLOTS):
            bi = RANKS * k + j
            out[b, bi * P:(bi + 1) * P, :] = o[k * P:(k + 1) * P, :]
    return out


# revision 27
# speedup vs baseline: 1.0428x; 1.0428x over previous
"""Trainium2 Bass kernel for nn_Attention_34351148434119 (8 NeuronCores).

Reference computation (faithful quirks included):
  q_proj = hid @ Wq; q, gate = split(q_proj)     # q is DEAD code downstream
  k = hid @ Wk; v = hid @ Wv                     # [B,KV,S,D]
  v = RoPE(v)  (k is NOT roped; q roped but unused)
  scores = (k @ v^T) * sqrt(D) + mask; attn = softmax_t(scores)   # per kv head
  out = (tile_G(attn @ v) * sigmoid(gate)) @ Wo

Sharding: core = b*4 + j  (b = batch, j = rank in 4-core batch group).
Per batch, S=2048 is split into 16 blocks of 128 rows; core j owns blocks
{j, 4+j, 8+j, 12+j} (slot k block = 4k+j) so every core has an identical
causal workload (uniform SPMD graph; per-core specialization only via
staged data).

Pipeline (v1 lessons: collectives starve host DMA queues while in
flight, and the PE clock ramps with uninterrupted streak length):
  1. v-projection only (fp32r: hw-internal bf16 hi/lo split at full PE
     speed), RoPE, stage, AllGather issued EARLY (~t=45us).
  2. k-projection (fp32r) from the resident hid tiles.
  3. Gate matmuls run bf16 from HOST-staged bf16 weights, fully
     preloaded into SBUF before the AG starts - zero DMA during the AG.
  4. Attention per kv head: fp32r scores (logits sigma~105: softmax is
     near-argmax, bf16 anywhere in k/v->scores flips rows), two-phase
     softmax, PE transposes, bf16 attn@v.  Row-major v is derived from
     the gathered d-major v by on-chip transposes (no 2nd AllGather).
  5. Gating + bf16 out-projection from host-staged bf16 Wo, streamed
     through a deep slab ring (no conversion ops).
"""
import sys
import numpy as np
import ml_dtypes

sys.path.insert(0, "/opt/trn_rl_repo")

B, S, HS = 2, 2048, 2048
H, KV, D = 16, 4, 128
G = H // KV
SCALING = float(D) ** 0.5
P = 128
NB = S // P            # 16 row blocks per batch
NCORES = 8
RANKS = 4              # cores per batch group
SLOTS = 4              # owned 128-row blocks per core
ROWS = SLOTS * P       # 512 rows per core
CHUNK = 512            # t-chunk = 4 t-tiles
NCHUNK = S // CHUNK    # 4
KT = HS // P           # 16 contraction tiles
KVD = KV * D
NEG_THRESH = -1e8

_CACHE = {}


def _mask_classes(mask):
    """Classify each (s-slot k, t-chunk c) 512x512 region of the SxS mask.

    0 = skip (everything <= NEG_THRESH: contributes exact 0 after softmax)
    1 = plain (all zeros: no add needed)
    2 = add  (mixed: stage values and add on-chip)
    Slot k rows across all cores = blocks 4k..4k+3 = rows [512k, 512k+512).
    """
    cls = [[0] * NCHUNK for _ in range(SLOTS)]
    for k in range(SLOTS):
        for c in range(NCHUNK):
            reg = mask[512 * k:512 * (k + 1), 512 * c:512 * (c + 1)]
            if (reg <= NEG_THRESH).all():
                cls[k][c] = 0
            elif (reg == 0).all():
                cls[k][c] = 1
            else:
                cls[k][c] = 2
    ok = True
    for k in range(SLOTS):
        comp = [c for c in range(NCHUNK) if cls[k][c] != 0]
        # computed chunks must be a prefix starting at 0
        if comp != list(range(len(comp))) or 0 not in comp:
            ok = False
    if ok:
        # {k : chunk c computed} must be a suffix of slots for each c
        for c in range(NCHUNK):
            ks = [k for k in range(SLOTS) if cls[k][c] != 0]
            if ks != list(range(SLOTS - len(ks), SLOTS)):
                ok = False
    if not ok:
        # fully dense fallback: always correct for any mask
        cls = [[2] * NCHUNK for _ in range(SLOTS)]
    return cls


def _mask_strips(mask, classes, j):
    """Per-core class-2 strips, in (k, c) scan order."""
    strips = []
    for k in range(SLOTS):
        for c in range(NCHUNK):
            if classes[k][c] == 2:
                bi = RANKS * k + j
                strips.append(np.ascontiguousarray(
                    mask[bi * P:(bi + 1) * P, c * CHUNK:(c + 1) * CHUNK]))
    return strips


def _dedup_map(mask, classes):
    """Map each class-2 (k,c) to a unique-strip index, valid for EVERY
    core (cores hold different rows, so strip equality must hold on all
    of them).  Returns (uniq_of_addidx, n_uniq) or None if coreswise
    inconsistent."""
    n_add = sum(1 for k in range(SLOTS) for c in range(NCHUNK)
                if classes[k][c] == 2)
    per_core = []
    for j in range(RANKS):
        strips = _mask_strips(mask, classes, j)
        uniq = []
        idx = []
        for s in strips:
            for ui, u in enumerate(uniq):
                if np.array_equal(s, u):
                    idx.append(ui)
                    break
            else:
                uniq.append(s)
                idx.append(len(uniq) - 1)
        per_core.append(tuple(idx))
    if len(set(per_core)) != 1:
        return tuple(range(n_add)), n_add     # no dedup
    return per_core[0], max(per_core[0]) + 1 if per_core[0] else 0


def _build(classes, uniq_idx, n_uniq):
    from contextlib import ExitStack

    from concourse import bacc, mybir, tile
    from concourse.masks import make_identity

    f32 = mybir.dt.float32
    f32r = mybir.dt.float32r
    bf16 = mybir.dt.bfloat16
    Alu = mybir.AluOpType
    Act = mybir.ActivationFunctionType

    computed = [[c for c in range(NCHUNK) if classes[k][c] != 0] for k in range(SLOTS)]
    add_idx = {}
    for k in range(SLOTS):
        for c in range(NCHUNK):
            if classes[k][c] == 2:
                add_idx[(k, c)] = len(add_idx)
    n_mask = max(n_uniq, 1)
    resident_mask = n_uniq <= 4

    nc = bacc.Bacc("TRN2", target_bir_lowering=False, debug=False,
                   num_devices=NCORES)

    hidT_d = nc.declare_dram_parameter("hidT", [HS, ROWS], f32r, isOutput=False)
    wqg_d = nc.declare_dram_parameter("wqg", [HS, HS], bf16, isOutput=False)
    wk_d = nc.declare_dram_parameter("wk", [HS, KVD], f32r, isOutput=False)
    wv_d = nc.declare_dram_parameter("wv", [HS, KVD], f32r, isOutput=False)
    wo_d = nc.declare_dram_parameter("wo", [HS, HS], bf16, isOutput=False)
    cosT_d = nc.declare_dram_parameter("cosT", [D, ROWS], f32, isOutput=False)
    sinT_d = nc.declare_dram_parameter("sinT", [D, ROWS], f32, isOutput=False)
    mask_d = nc.declare_dram_parameter("maskst", [n_mask, P, CHUNK], f32,
                                       isOutput=False)
    out_d = nc.declare_dram_parameter("out", [ROWS, HS], f32, isOutput=True)

    rg = [[0, 1, 2, 3], [4, 5, 6, 7]]
    NSLAB = 64    # bf16 weight-slab ring (full gate preload, wo streams through)

    with tile.TileContext(nc) as tc, ExitStack() as ctx:
        sb = ctx.enter_context(tc.tile_pool(name="sb", bufs=2))
        ps = ctx.enter_context(tc.tile_pool(name="ps", bufs=8, space="PSUM"))
        dram = ctx.enter_context(tc.tile_pool(name="dram", bufs=1, space="DRAM"))

        # ---- constants ----
        id_f32 = sb.tile([P, P], f32, tag="c_idf")
        id_bf = sb.tile([P, P], bf16, tag="c_idb")
        make_identity(nc, id_f32[:])
        make_identity(nc, id_bf[:])

        # ---- combined k+v projection (fp32r): the front half is at the
        # aggregate HBM roofline, so k matmuls ride the same DMA-bound
        # window as v instead of consuming PE time afterwards ----
        pv = [ps.tile([P, ROWS], f32, tag="ps", name=f"pv{g}") for g in range(KV)]
        pk = [ps.tile([P, ROWS], f32, tag="ps", name=f"pk{g}") for g in range(KV)]
        hidb = []
        hfw = [sb.tile([P, 4 * ROWS], f32r, tag="f32big", bufs=2, name=f"hfw{q}")
               for q in range(4)]
        hid = [hfw[kk // 4][:, (kk % 4) * ROWS:(kk % 4 + 1) * ROWS]
               for kk in range(KT)]
        for kk in range(KT):
            nc.sync.dma_start(hid[kk], hidT_d[kk * P:(kk + 1) * P, :])
            wt = sb.tile([P, 2 * KVD], f32r, tag="wkv", bufs=2, name=f"wv{kk}")
            nc.gpsimd.dma_start(wt[:, :KVD], wv_d[kk * P:(kk + 1) * P, :])
            nc.gpsimd.dma_start(wt[:, KVD:], wk_d[kk * P:(kk + 1) * P, :])
            for g in range(KV):
                nc.tensor.matmul(pv[g][:], wt[:, g * P:(g + 1) * P], hid[kk],
                                 start=(kk == 0), stop=(kk == KT - 1))
            for g in range(KV):
                nc.tensor.matmul(pk[g][:], wt[:, KVD + g * P:KVD + (g + 1) * P],
                                 hid[kk], start=(kk == 0), stop=(kk == KT - 1))
            hb = sb.tile([P, ROWS], bf16, tag="hidb", bufs=16, name=f"hb{kk}")
            nc.scalar.copy(hb[:], hid[kk].bitcast(f32))
            hidb.append(hb)

        # ---- cos/sin/mask after the kv stream (needed only ~t=75) ----
        cosT = sb.tile([D, ROWS], f32, tag="c_cos")
        sinT = sb.tile([D, ROWS], f32, tag="c_sin")
        nc.sync.dma_start(cosT[:], cosT_d[:, :])
        nc.sync.dma_start(sinT[:], sinT_d[:, :])
        mtiles = []
        if resident_mask:
            for u in range(n_uniq):
                mt = sb.tile([P, CHUNK], f32, tag="msk", bufs=max(n_uniq, 1),
                             name=f"mt{u}")
                nc.sync.dma_start(mt[:], mask_d[u, :, :])
                mtiles.append(mt)

        # ---- gate weights: wide slabs on the SCALAR dma queue ----
        wqb = []
        for kk in range(KT):
            t = sb.tile([P, HS], bf16, tag="wslab", bufs=16, name=f"wq{kk}")
            nc.scalar.dma_start(t[:], wqg_d[kk * P:(kk + 1) * P, :])
            wqb.append(t)

        # ---- RoPE v pre-AG; per-g AllGather (4 small collectives) so
        # head 0's data lands well before attention needs it ----
        vt_in = dram.tile([KVD, ROWS], f32)
        vt_all = [dram.tile([RANKS * P, ROWS], f32, name=f"vtall{g}")
                  for g in range(KV)]
        for g in range(KV):
            vr = sb.tile([P, ROWS], f32, tag="vraw", bufs=2, name=f"vr{g}")
            nc.scalar.copy(vr[:], pv[g][:])
            rot = sb.tile([P, ROWS], f32, tag="vrot", bufs=2, name=f"rot{g}")
            nc.vector.tensor_scalar_mul(rot[0:64, :], vr[64:128, :], -1.0)
            nc.vector.tensor_copy(rot[64:128, :], vr[0:64, :])
            nc.vector.tensor_mul(vr[:], vr[:], cosT[:])
            nc.vector.tensor_mul(rot[:], rot[:], sinT[:])
            nc.vector.tensor_add(vr[:], vr[:], rot[:])
            nc.gpsimd.dma_start(vt_in[g * P:(g + 1) * P, :], vr[:])
            nc.gpsimd.collective_compute(
                "AllGather", mybir.AluOpType.bypass, replica_groups=rg,
                ins=[vt_in[g * P:(g + 1) * P, :].opt()],
                outs=[vt_all[g].opt()])

        kT = []   # per g: [128 d, 512 rows] f32r, pre-scaled by sqrt(D)
        for g in range(KV):
            t = sb.tile([P, ROWS], f32r, tag="kT", bufs=4, name=f"kT{g}")
            nc.scalar.mul(t[:], pk[g][:], SCALING)
            kT.append(t)

        # ---- gathered v: per-g wide tiles recycling the hid ring
        # (bufs=2).  g=0,1 load right after the AG; g=2,3 are deferred
        # into the attention pipeline so their ring waits never block
        # the queue mid-phase. ----
        vtcg = []

        def emit_vtc_load(g):
            t = vtcg[g]
            for c in range(NCHUNK):
                for r in range(RANKS):
                    nc.sync.dma_start(
                        t[:, (4 * c + r) * P:(4 * c + r + 1) * P],
                        vt_all[g][r * P:(r + 1) * P,
                                  c * P:(c + 1) * P].bitcast(f32r))

        for g in range(KV):
            vtcg.append(sb.tile([P, 4 * CHUNK], f32r, tag="f32big", bufs=2,
                                name=f"vtcg{g}"))
        emit_vtc_load(0)
        emit_vtc_load(1)

        # ---- gate matmul (bf16) + fused sigmoid: zero DMA during AG;
        # the last 6 groups interleave with head-0 attention so the PE
        # covers the softmax chain ----
        sigT = [None] * 16

        def emit_gate(nblk, m):
            pg = ps.tile([P, ROWS], f32, tag="ps", name=f"pg{nblk}_{m}")
            for kk in range(KT):
                nc.tensor.matmul(
                    pg[:], wqb[kk][:, nblk * CHUNK + m * P:nblk * CHUNK + (m + 1) * P],
                    hidb[kk][:], start=(kk == 0), stop=(kk == KT - 1))
            t = sb.tile([P, ROWS], bf16, tag="sigT", bufs=16, name=f"sig{nblk}_{m}")
            nc.scalar.activation(t[:], pg[:], Act.Sigmoid)
            sigT[nblk * 4 + m] = t

        # ---- attention: flat software pipeline over (g, slot) tasks.
        # scores of slot k+1/k+2 and transposes of slot k-1 overlap the
        # Act/DVE softmax chain of slot k, so the PE never drains. ----
        avT = [None] * KV
        vrows = {}
        attnTs = {}
        sstate = {}
        tstate = {}
        pavs = {}
        def emit_vrow(g):
            vl = []
            for bi in range(NB):
                tp = ps.tile([P, P], f32, tag="ps", name=f"tvp{g}_{bi}")
                nc.tensor.transpose(
                    tp[:], vtcg[g][:, bi * P:(bi + 1) * P].bitcast(f32), id_f32[:])
                t = sb.tile([P, P], bf16, tag="vrow", bufs=32,
                            name=f"vrow{g}_{bi}")
                if bi % 2:
                    nc.scalar.copy(t[:], tp[:])
                else:
                    nc.vector.tensor_copy(t[:], tp[:])
                vl.append(t)
            vrows[g] = vl
            attnTs[g] = [sb.tile([P, ROWS], bf16, tag="attnT", bufs=16,
                                 name=f"attnT{g}_{bi}") for bi in range(NB)]

        def emit_scores(g, k):
            comp = computed[k]
            pscs = []
            cms = []
            for ci, c in enumerate(comp):
                psc = ps.tile([P, CHUNK], f32, tag="ps", name=f"psc{g}_{k}_{ci}")
                nc.tensor.matmul(psc[:], kT[g][:, k * P:(k + 1) * P],
                                 vtcg[g][:, c * CHUNK:(c + 1) * CHUNK],
                                 start=True, stop=True)
                if classes[k][c] == 2:
                    ai = add_idx[(k, c)]
                    if resident_mask:
                        mt = mtiles[uniq_idx[ai]]
                    else:
                        mt = sb.tile([P, CHUNK], f32, tag="msk", bufs=4,
                                     name=f"msk{g}_{k}_{c}")
                        nc.gpsimd.dma_start(mt[:], mask_d[ai, :, :])
                    nc.vector.tensor_add(psc[:], psc[:], mt[:])
                cm = sb.tile([P, 1], f32, tag="stat", bufs=32,
                             name=f"cm{g}_{k}_{ci}")
                nc.vector.tensor_reduce(cm[:], psc[:], mybir.AxisListType.X,
                                        Alu.max, negate=True)
                pscs.append(psc)
                cms.append(cm)
            sstate[(g, k)] = (pscs, cms, comp)

        def emit_softmax(g, k):
            pscs, cms, comp = sstate.pop((g, k))
            nchk = len(comp)
            attn = sb.tile([P, CHUNK * nchk], bf16, tag="attn", bufs=2,
                           padded_shape=[P, CHUNK * NCHUNK],
                           name=f"attn{g}_{k}")
            mneg = cms[0]   # -max
            for ci in range(1, nchk):
                mnew = sb.tile([P, 1], f32, tag="stat", bufs=32,
                               name=f"mn{g}_{k}_{ci}")
                nc.vector.tensor_tensor(mnew[:], mneg[:], cms[ci][:], Alu.min)
                mneg = mnew
            tot = None
            for ci in range(nchk):
                csum = sb.tile([P, 1], f32, tag="stat", bufs=32,
                               name=f"cs{g}_{k}_{ci}")
                nc.scalar.activation(attn[:, ci * CHUNK:(ci + 1) * CHUNK],
                                     pscs[ci][:], Act.Exp, bias=mneg[:],
                                     accum_out=csum[:])
                if tot is None:
                    tot = csum
                else:
                    t2 = sb.tile([P, 1], f32, tag="stat", bufs=32,
                                 name=f"tt{g}_{k}_{ci}")
                    nc.vector.tensor_add(t2[:], tot[:], csum[:])
                    tot = t2
            rinv = sb.tile([P, 1], f32, tag="stat", bufs=32,
                           name=f"ri{g}_{k}")
            nc.vector.reciprocal(rinv[:], tot[:])
            for ci in range(nchk):
                nc.vector.tensor_scalar_mul(
                    attn[:, ci * CHUNK:(ci + 1) * CHUNK],
                    attn[:, ci * CHUNK:(ci + 1) * CHUNK], rinv[:])
            tstate[(g, k)] = (attn, comp)

        def emit_transposes(g, k):
            attn, comp = tstate.pop((g, k))
            attnT = attnTs[g]
            for ci, c in enumerate(comp):
                for i in range(4):
                    bi = 4 * c + i
                    tp = ps.tile([P, P], bf16, tag="ps", name=f"tap{g}_{k}_{bi}")
                    nc.tensor.transpose(
                        tp[:], attn[:, ci * CHUNK + i * P:ci * CHUNK + (i + 1) * P],
                        id_bf[:])
                    if i % 2:
                        nc.scalar.copy(attnT[bi][:, k * P:(k + 1) * P], tp[:])
                    else:
                        nc.vector.tensor_copy(attnT[bi][:, k * P:(k + 1) * P], tp[:])

        def emit_av_mm(g):
            pav = ps.tile([P, ROWS], f32, tag="ps", name=f"pav{g}")
            first = True
            for bi in range(NB):
                ks = [k for k in range(SLOTS) if (bi // RANKS) in computed[k]]
                if not ks:
                    continue
                kmin = ks[0]
                nc.tensor.matmul(pav[:, kmin * P:ROWS], vrows[g][bi][:],
                                 attnTs[g][bi][:, kmin * P:ROWS],
                                 start=first, stop=(bi == NB - 1))
                first = False
            pavs[g] = pav

        def emit_av_drain(g):
            t = sb.tile([P, ROWS], bf16, tag="avT", bufs=4, name=f"avT{g}")
            nc.vector.tensor_copy(t[:], pavs.pop(g)[:])
            avT[g] = t
            for i in range(G):
                s = sigT[4 * g + i]
                nc.vector.tensor_mul(s[:], t[:], s[:])

        gg = [(nblk, m) for nblk in range(4) for m in range(4)]
        for j in range(10):
            emit_gate(*gg[j])
        emit_vrow(0)
        emit_gate(*gg[10])
        emit_scores(0, 0)
        emit_scores(0, 1)
        emit_gate(*gg[11])
        emit_softmax(0, 0)
        emit_scores(0, 2)
        emit_gate(*gg[12])
        emit_softmax(0, 1)
        emit_transposes(0, 0)
        emit_gate(*gg[13])
        emit_scores(0, 3)
        emit_softmax(0, 2)
        emit_transposes(0, 1)
        emit_gate(*gg[14])
        emit_vrow(1)
        emit_softmax(0, 3)
        emit_transposes(0, 2)
        emit_gate(*gg[15])
        emit_transposes(0, 3)
        emit_av_mm(0)
        emit_vtc_load(2)
        emit_scores(1, 0)
        emit_scores(1, 1)
        emit_av_drain(0)
        for g in range(1, KV):
            emit_softmax(g, 0)
            emit_scores(g, 2)
            emit_softmax(g, 1)
            emit_transposes(g, 0)
            emit_scores(g, 3)
            emit_softmax(g, 2)
            emit_transposes(g, 1)
            emit_softmax(g, 3)
            emit_transposes(g, 2)
            emit_transposes(g, 3)
            emit_av_mm(g)
            if g + 2 < KV:
                emit_vtc_load(g + 2)
            if g + 1 < KV:
                emit_vrow(g + 1)
                emit_scores(g + 1, 0)
                emit_scores(g + 1, 1)
            emit_av_drain(g)

        gat = sigT   # gating applied in-place per-g inside the pipeline

        # ---- out projection (bf16, host-staged wide slabs, ring reuse) ----
        wob = []
        for cc in range(KT):
            t = sb.tile([P, HS], bf16, tag="wslab", bufs=16, name=f"wo{cc}")
            nc.sync.dma_start(t[:], wo_d[cc * P:(cc + 1) * P, :])
            wob.append(t)
        for nblk in range(4):
            for rt in range(SLOTS):
                po = ps.tile([P, CHUNK], f32, tag="ps")
                for cc in range(KT):
                    nc.tensor.matmul(
                        po[:], gat[cc][:, rt * P:(rt + 1) * P],
                        wob[cc][:, nblk * CHUNK:(nblk + 1) * CHUNK],
                        start=(cc == 0), stop=(cc == KT - 1))
                t = sb.tile([P, CHUNK], f32, tag="oev", bufs=2)
                nc.scalar.copy(t[:], po[:])
                nc.sync.dma_start(
                    out_d[rt * P:(rt + 1) * P, nblk * CHUNK:(nblk + 1) * CHUNK], t[:])

    nc.compile()
    return nc


def kernel(hidden_states, cos, sin, attention_mask, Wq, Wk, Wv, Wo):
    from concourse.bass_utils import run_bass_kernel_spmd

    hidden_states = np.asarray(hidden_states, dtype=np.float32)
    cos = np.asarray(cos, dtype=np.float32)
    sin = np.asarray(sin, dtype=np.float32)
    mask = np.asarray(attention_mask, dtype=np.float32)[0, 0]
    Wq = np.asarray(Wq, dtype=np.float32)
    Wk = np.asarray(Wk, dtype=np.float32)
    Wv = np.asarray(Wv, dtype=np.float32)
    Wo = np.asarray(Wo, dtype=np.float32)

    classes = _mask_classes(mask)
    uniq_idx, n_uniq = _dedup_map(mask, classes)
    key = (tuple(tuple(r) for r in classes), tuple(uniq_idx), n_uniq)
    if key not in _CACHE:
        _CACHE[key] = _build(classes, uniq_idx, n_uniq)
    nc = _CACHE[key]

    wqg = np.ascontiguousarray(Wq[:, HS:]).astype(ml_dtypes.bfloat16)
    wob = Wo.astype(ml_dtypes.bfloat16)

    in_maps = []
    for core in range(NCORES):
        b, j = divmod(core, RANKS)
        blocks = [RANKS * k + j for k in range(SLOTS)]
        rows = np.concatenate([np.arange(bi * P, (bi + 1) * P) for bi in blocks])
        strips = _mask_strips(mask, classes, j)
        if n_uniq > 0 and len(set(uniq_idx)) != len(strips):
            # staged per unique index
            uniq_strips = [None] * (max(uniq_idx) + 1)
            for si, ui in enumerate(uniq_idx):
                if uniq_strips[ui] is None:
                    uniq_strips[ui] = strips[si]
            strips = uniq_strips
        if not strips:
            strips = [np.zeros((P, CHUNK), np.float32)]
        hidT = np.ascontiguousarray(hidden_states[b][rows].T)
        in_maps.append({
            "hidT": hidT,
            "wqg": wqg,
            "wk": Wk,
            "wv": Wv,
            "wo": wob,
            "cosT": np.ascontiguousarray(cos[b][rows].T),
            "sinT": np.ascontiguousarray(sin[b][rows].T),
            "maskst": np.ascontiguousarray(np.stack(strips)),
        })

    res = run_bass_kernel_spmd(nc, in_maps, core_ids=list(range(NCORES)))

    out = np.empty((B, S, HS), np.float32)
    for core in range(NCORES):
        b, j = divmod(core, RANKS)
        o = res.results[core]["out"]
        for k in range(SLOTS):
            bi = RANKS * k + j
            out[b, bi * P:(bi + 1) * P, :] = o[k * P:(k + 1) * P, :]
    return out


# revision 28
# speedup vs baseline: 1.0795x; 1.0352x over previous
"""Trainium2 Bass kernel for nn_Attention_34351148434119 (8 NeuronCores).

Reference computation (faithful quirks included):
  q_proj = hid @ Wq; q, gate = split(q_proj)     # q is DEAD code downstream
  k = hid @ Wk; v = hid @ Wv                     # [B,KV,S,D]
  v = RoPE(v)  (k is NOT roped; q roped but unused)
  scores = (k @ v^T) * sqrt(D) + mask; attn = softmax_t(scores)   # per kv head
  out = (tile_G(attn @ v) * sigmoid(gate)) @ Wo

Sharding: core = b*4 + j  (b = batch, j = rank in 4-core batch group).
Per batch, S=2048 is split into 16 blocks of 128 rows; core j owns blocks
{j, 4+j, 8+j, 12+j} (slot k block = 4k+j) so every core has an identical
causal workload (uniform SPMD graph; per-core specialization only via
staged data).

Pipeline (v1 lessons: collectives starve host DMA queues while in
flight, and the PE clock ramps with uninterrupted streak length):
  1. v-projection only (fp32r: hw-internal bf16 hi/lo split at full PE
     speed), RoPE, stage, AllGather issued EARLY (~t=45us).
  2. k-projection (fp32r) from the resident hid tiles.
  3. Gate matmuls run bf16 from HOST-staged bf16 weights, fully
     preloaded into SBUF before the AG starts - zero DMA during the AG.
  4. Attention per kv head: fp32r scores (logits sigma~105: softmax is
     near-argmax, bf16 anywhere in k/v->scores flips rows), two-phase
     softmax, PE transposes, bf16 attn@v.  Row-major v is derived from
     the gathered d-major v by on-chip transposes (no 2nd AllGather).
  5. Gating + bf16 out-projection from host-staged bf16 Wo, streamed
     through a deep slab ring (no conversion ops).
"""
import sys
import numpy as np
import ml_dtypes

sys.path.insert(0, "/opt/trn_rl_repo")

B, S, HS = 2, 2048, 2048
H, KV, D = 16, 4, 128
G = H // KV
SCALING = float(D) ** 0.5
P = 128
NB = S // P            # 16 row blocks per batch
NCORES = 8
RANKS = 4              # cores per batch group
SLOTS = 4              # owned 128-row blocks per core
ROWS = SLOTS * P       # 512 rows per core
CHUNK = 512            # t-chunk = 4 t-tiles
NCHUNK = S // CHUNK    # 4
KT = HS // P           # 16 contraction tiles
KVD = KV * D
NEG_THRESH = -1e8

_CACHE = {}


def _mask_classes(mask):
    """Classify each (s-slot k, t-chunk c) 512x512 region of the SxS mask.

    0 = skip (everything <= NEG_THRESH: contributes exact 0 after softmax)
    1 = plain (all zeros: no add needed)
    2 = add  (mixed: stage values and add on-chip)
    Slot k rows across all cores = blocks 4k..4k+3 = rows [512k, 512k+512).
    """
    cls = [[0] * NCHUNK for _ in range(SLOTS)]
    for k in range(SLOTS):
        for c in range(NCHUNK):
            reg = mask[512 * k:512 * (k + 1), 512 * c:512 * (c + 1)]
            if (reg <= NEG_THRESH).all():
                cls[k][c] = 0
            elif (reg == 0).all():
                cls[k][c] = 1
            else:
                cls[k][c] = 2
    ok = True
    for k in range(SLOTS):
        comp = [c for c in range(NCHUNK) if cls[k][c] != 0]
        # computed chunks must be a prefix starting at 0
        if comp != list(range(len(comp))) or 0 not in comp:
            ok = False
    if ok:
        # {k : chunk c computed} must be a suffix of slots for each c
        for c in range(NCHUNK):
            ks = [k for k in range(SLOTS) if cls[k][c] != 0]
            if ks != list(range(SLOTS - len(ks), SLOTS)):
                ok = False
    if not ok:
        # fully dense fallback: always correct for any mask
        cls = [[2] * NCHUNK for _ in range(SLOTS)]
    return cls


def _mask_strips(mask, classes, j):
    """Per-core class-2 strips, in (k, c) scan order."""
    strips = []
    for k in range(SLOTS):
        for c in range(NCHUNK):
            if classes[k][c] == 2:
                bi = RANKS * k + j
                strips.append(np.ascontiguousarray(
                    mask[bi * P:(bi + 1) * P, c * CHUNK:(c + 1) * CHUNK]))
    return strips


def _dedup_map(mask, classes):
    """Map each class-2 (k,c) to a unique-strip index, valid for EVERY
    core (cores hold different rows, so strip equality must hold on all
    of them).  Returns (uniq_of_addidx, n_uniq) or None if coreswise
    inconsistent."""
    n_add = sum(1 for k in range(SLOTS) for c in range(NCHUNK)
                if classes[k][c] == 2)
    per_core = []
    for j in range(RANKS):
        strips = _mask_strips(mask, classes, j)
        uniq = []
        idx = []
        for s in strips:
            for ui, u in enumerate(uniq):
                if np.array_equal(s, u):
                    idx.append(ui)
                    break
            else:
                uniq.append(s)
                idx.append(len(uniq) - 1)
        per_core.append(tuple(idx))
    if len(set(per_core)) != 1:
        return tuple(range(n_add)), n_add     # no dedup
    return per_core[0], max(per_core[0]) + 1 if per_core[0] else 0


def _build(classes, uniq_idx, n_uniq):
    from contextlib import ExitStack

    from concourse import bacc, mybir, tile
    from concourse.masks import make_identity

    f32 = mybir.dt.float32
    f32r = mybir.dt.float32r
    bf16 = mybir.dt.bfloat16
    Alu = mybir.AluOpType
    Act = mybir.ActivationFunctionType

    computed = [[c for c in range(NCHUNK) if classes[k][c] != 0] for k in range(SLOTS)]
    add_idx = {}
    for k in range(SLOTS):
        for c in range(NCHUNK):
            if classes[k][c] == 2:
                add_idx[(k, c)] = len(add_idx)
    n_mask = max(n_uniq, 1)
    resident_mask = n_uniq <= 4

    nc = bacc.Bacc("TRN2", target_bir_lowering=False, debug=False,
                   num_devices=NCORES)

    hidT_d = nc.declare_dram_parameter("hidT", [HS, ROWS], f32r, isOutput=False)
    wqg_d = nc.declare_dram_parameter("wqg", [HS, HS], bf16, isOutput=False)
    wk_d = nc.declare_dram_parameter("wk", [HS, KVD], f32r, isOutput=False)
    wv_d = nc.declare_dram_parameter("wv", [HS, KVD], f32r, isOutput=False)
    wo_d = nc.declare_dram_parameter("wo", [HS, HS], bf16, isOutput=False)
    cosT_d = nc.declare_dram_parameter("cosT", [D, ROWS], f32, isOutput=False)
    sinT_d = nc.declare_dram_parameter("sinT", [D, ROWS], f32, isOutput=False)
    mask_d = nc.declare_dram_parameter("maskst", [n_mask, P, CHUNK], f32,
                                       isOutput=False)
    out_d = nc.declare_dram_parameter("out", [ROWS, HS], f32, isOutput=True)

    rg = [[0, 1, 2, 3], [4, 5, 6, 7]]
    NSLAB = 64    # bf16 weight-slab ring (full gate preload, wo streams through)

    with tile.TileContext(nc) as tc, ExitStack() as ctx:
        sb = ctx.enter_context(tc.tile_pool(name="sb", bufs=2))
        ps = ctx.enter_context(tc.tile_pool(name="ps", bufs=8, space="PSUM"))
        dram = ctx.enter_context(tc.tile_pool(name="dram", bufs=1, space="DRAM"))

        # ---- constants ----
        id_f32 = sb.tile([P, P], f32, tag="c_idf")
        id_bf = sb.tile([P, P], bf16, tag="c_idb")
        make_identity(nc, id_f32[:])
        make_identity(nc, id_bf[:])

        # ---- combined k+v projection (fp32r): the front half is at the
        # aggregate HBM roofline, so k matmuls ride the same DMA-bound
        # window as v instead of consuming PE time afterwards ----
        pv = [ps.tile([P, ROWS], f32, tag="ps", name=f"pv{g}") for g in range(KV)]
        pk = [ps.tile([P, ROWS], f32, tag="ps", name=f"pk{g}") for g in range(KV)]
        hidb = []
        hfw = [sb.tile([P, 4 * ROWS], f32r, tag="f32big", bufs=2, name=f"hfw{q}")
               for q in range(4)]
        hid = [hfw[kk // 4][:, (kk % 4) * ROWS:(kk % 4 + 1) * ROWS]
               for kk in range(KT)]
        for kk in range(KT):
            nc.sync.dma_start(hid[kk], hidT_d[kk * P:(kk + 1) * P, :])
            wt = sb.tile([P, 2 * KVD], f32r, tag="wkv", bufs=2, name=f"wv{kk}")
            nc.gpsimd.dma_start(wt[:, :KVD], wv_d[kk * P:(kk + 1) * P, :])
            nc.gpsimd.dma_start(wt[:, KVD:], wk_d[kk * P:(kk + 1) * P, :])
            for g in range(KV):
                nc.tensor.matmul(pv[g][:], wt[:, g * P:(g + 1) * P], hid[kk],
                                 start=(kk == 0), stop=(kk == KT - 1))
            for g in range(KV):
                nc.tensor.matmul(pk[g][:], wt[:, KVD + g * P:KVD + (g + 1) * P],
                                 hid[kk], start=(kk == 0), stop=(kk == KT - 1))
            hb = sb.tile([P, ROWS], bf16, tag="hidb", bufs=16, name=f"hb{kk}")
            nc.scalar.copy(hb[:], hid[kk].bitcast(f32))
            hidb.append(hb)

        # ---- cos/sin/mask after the kv stream (needed only ~t=75) ----
        cosT = sb.tile([D, ROWS], f32, tag="c_cos")
        sinT = sb.tile([D, ROWS], f32, tag="c_sin")
        nc.sync.dma_start(cosT[:], cosT_d[:, :])
        nc.sync.dma_start(sinT[:], sinT_d[:, :])
        mtiles = []
        if resident_mask:
            for u in range(n_uniq):
                mt = sb.tile([P, CHUNK], f32, tag="msk", bufs=max(n_uniq, 1),
                             name=f"mt{u}")
                nc.sync.dma_start(mt[:], mask_d[u, :, :])
                mtiles.append(mt)

        # ---- gate weights: wide slabs on the SCALAR dma queue ----
        wqb = []
        for kk in range(KT):
            t = sb.tile([P, HS], bf16, tag="wslab", bufs=16, name=f"wq{kk}")
            nc.scalar.dma_start(t[:], wqg_d[kk * P:(kk + 1) * P, :])
            wqb.append(t)

        # ---- RoPE v pre-AG; per-g AllGather (4 small collectives) so
        # head 0's data lands well before attention needs it ----
        vt_in = dram.tile([KVD, ROWS], f32)
        vt_all = [dram.tile([RANKS * P, ROWS], f32, name=f"vtall{g}")
                  for g in range(KV)]
        for g in range(KV):
            vr = sb.tile([P, ROWS], f32, tag="vraw", bufs=2, name=f"vr{g}")
            nc.scalar.copy(vr[:], pv[g][:])
            rot = sb.tile([P, ROWS], f32, tag="vrot", bufs=2, name=f"rot{g}")
            nc.vector.tensor_scalar_mul(rot[0:64, :], vr[64:128, :], -1.0)
            nc.vector.tensor_copy(rot[64:128, :], vr[0:64, :])
            nc.vector.tensor_mul(vr[:], vr[:], cosT[:])
            nc.vector.tensor_mul(rot[:], rot[:], sinT[:])
            nc.vector.tensor_add(vr[:], vr[:], rot[:])
            nc.gpsimd.dma_start(vt_in[g * P:(g + 1) * P, :], vr[:])
            nc.gpsimd.collective_compute(
                "AllGather", mybir.AluOpType.bypass, replica_groups=rg,
                ins=[vt_in[g * P:(g + 1) * P, :].opt()],
                outs=[vt_all[g].opt()])

        kT = []   # per g: [128 d, 512 rows] f32r, pre-scaled by sqrt(D)
        for g in range(KV):
            t = sb.tile([P, ROWS], f32r, tag="kT", bufs=4, name=f"kT{g}")
            nc.scalar.mul(t[:], pk[g][:], SCALING)
            kT.append(t)

        # ---- gathered v: per-g wide tiles recycling the hid ring
        # (bufs=2).  g=0,1 load right after the AG; g=2,3 are deferred
        # into the attention pipeline so their ring waits never block
        # the queue mid-phase. ----
        vtcg = []

        def emit_vtc_load(g):
            t = vtcg[g]
            for c in range(NCHUNK):
                for r in range(RANKS):
                    nc.sync.dma_start(
                        t[:, (4 * c + r) * P:(4 * c + r + 1) * P],
                        vt_all[g][r * P:(r + 1) * P,
                                  c * P:(c + 1) * P].bitcast(f32r))

        for g in range(KV):
            vtcg.append(sb.tile([P, 4 * CHUNK], f32r, tag="f32big", bufs=2,
                                name=f"vtcg{g}"))
        emit_vtc_load(0)
        emit_vtc_load(1)

        # ---- gate matmul (bf16) + fused sigmoid: zero DMA during AG;
        # the last 6 groups interleave with head-0 attention so the PE
        # covers the softmax chain ----
        sigT = [None] * 16

        def emit_gate(nblk, m):
            pg = ps.tile([P, ROWS], f32, tag="ps", name=f"pg{nblk}_{m}")
            for kk in range(KT):
                nc.tensor.matmul(
                    pg[:], wqb[kk][:, nblk * CHUNK + m * P:nblk * CHUNK + (m + 1) * P],
                    hidb[kk][:], start=(kk == 0), stop=(kk == KT - 1))
            t = sb.tile([P, ROWS], bf16, tag="sigT", bufs=16, name=f"sig{nblk}_{m}")
            nc.scalar.activation(t[:], pg[:], Act.Sigmoid)
            sigT[nblk * 4 + m] = t

        # ---- attention: flat software pipeline over (g, slot) tasks.
        # scores of slot k+1/k+2 and transposes of slot k-1 overlap the
        # Act/DVE softmax chain of slot k, so the PE never drains. ----
        avT = [None] * KV
        vrows = {}
        attnTs = {}
        sstate = {}
        tstate = {}
        pavs = {}
        def emit_vrow(g):
            vl = []
            for bi in range(NB):
                tp = ps.tile([P, P], f32, tag="ps", name=f"tvp{g}_{bi}")
                nc.tensor.transpose(
                    tp[:], vtcg[g][:, bi * P:(bi + 1) * P].bitcast(f32), id_f32[:])
                t = sb.tile([P, P], bf16, tag="vrow", bufs=32,
                            name=f"vrow{g}_{bi}")
                if bi % 2:
                    nc.scalar.copy(t[:], tp[:])
                else:
                    nc.vector.tensor_copy(t[:], tp[:])
                vl.append(t)
            vrows[g] = vl
            attnTs[g] = [sb.tile([P, ROWS], bf16, tag="attnT", bufs=16,
                                 name=f"attnT{g}_{bi}") for bi in range(NB)]

        def emit_scores(g, k):
            comp = computed[k]
            pscs = []
            cms = []
            for ci, c in enumerate(comp):
                psc = ps.tile([P, CHUNK], f32, tag="ps", name=f"psc{g}_{k}_{ci}")
                nc.tensor.matmul(psc[:], kT[g][:, k * P:(k + 1) * P],
                                 vtcg[g][:, c * CHUNK:(c + 1) * CHUNK],
                                 start=True, stop=True)
                if classes[k][c] == 2:
                    ai = add_idx[(k, c)]
                    if resident_mask:
                        mt = mtiles[uniq_idx[ai]]
                    else:
                        mt = sb.tile([P, CHUNK], f32, tag="msk", bufs=4,
                                     name=f"msk{g}_{k}_{c}")
                        nc.gpsimd.dma_start(mt[:], mask_d[ai, :, :])
                    nc.vector.tensor_add(psc[:], psc[:], mt[:])
                cm = sb.tile([P, 1], f32, tag="stat", bufs=32,
                             name=f"cm{g}_{k}_{ci}")
                nc.vector.tensor_reduce(cm[:], psc[:], mybir.AxisListType.X,
                                        Alu.max, negate=True)
                pscs.append(psc)
                cms.append(cm)
            sstate[(g, k)] = (pscs, cms, comp)

        def emit_softmax(g, k):
            pscs, cms, comp = sstate.pop((g, k))
            nchk = len(comp)
            attn = sb.tile([P, CHUNK * nchk], bf16, tag="attn", bufs=3,
                           padded_shape=[P, CHUNK * NCHUNK],
                           name=f"attn{g}_{k}")
            mneg = cms[0]   # -max
            for ci in range(1, nchk):
                mnew = sb.tile([P, 1], f32, tag="stat", bufs=32,
                               name=f"mn{g}_{k}_{ci}")
                nc.vector.tensor_tensor(mnew[:], mneg[:], cms[ci][:], Alu.min)
                mneg = mnew
            tot = None
            for ci in range(nchk):
                csum = sb.tile([P, 1], f32, tag="stat", bufs=32,
                               name=f"cs{g}_{k}_{ci}")
                nc.scalar.activation(attn[:, ci * CHUNK:(ci + 1) * CHUNK],
                                     pscs[ci][:], Act.Exp, bias=mneg[:],
                                     accum_out=csum[:])
                if tot is None:
                    tot = csum
                else:
                    t2 = sb.tile([P, 1], f32, tag="stat", bufs=32,
                                 name=f"tt{g}_{k}_{ci}")
                    nc.vector.tensor_add(t2[:], tot[:], csum[:])
                    tot = t2
            rinv = sb.tile([P, 1], f32, tag="stat", bufs=32,
                           name=f"ri{g}_{k}")
            nc.vector.reciprocal(rinv[:], tot[:])
            for ci in range(nchk):
                nc.vector.tensor_scalar_mul(
                    attn[:, ci * CHUNK:(ci + 1) * CHUNK],
                    attn[:, ci * CHUNK:(ci + 1) * CHUNK], rinv[:])
            tstate[(g, k)] = (attn, comp)

        def emit_transposes(g, k):
            attn, comp = tstate.pop((g, k))
            attnT = attnTs[g]
            for ci, c in enumerate(comp):
                for i in range(4):
                    bi = 4 * c + i
                    tp = ps.tile([P, P], bf16, tag="ps", name=f"tap{g}_{k}_{bi}")
                    nc.tensor.transpose(
                        tp[:], attn[:, ci * CHUNK + i * P:ci * CHUNK + (i + 1) * P],
                        id_bf[:])
                    if i % 2:
                        nc.scalar.copy(attnT[bi][:, k * P:(k + 1) * P], tp[:])
                    else:
                        nc.vector.tensor_copy(attnT[bi][:, k * P:(k + 1) * P], tp[:])

        def emit_av_mm(g):
            pav = ps.tile([P, ROWS], f32, tag="ps", name=f"pav{g}")
            first = True
            for bi in range(NB):
                ks = [k for k in range(SLOTS) if (bi // RANKS) in computed[k]]
                if not ks:
                    continue
                kmin = ks[0]
                nc.tensor.matmul(pav[:, kmin * P:ROWS], vrows[g][bi][:],
                                 attnTs[g][bi][:, kmin * P:ROWS],
                                 start=first, stop=(bi == NB - 1))
                first = False
            pavs[g] = pav

        def emit_av_drain(g):
            t = sb.tile([P, ROWS], bf16, tag="avT", bufs=4, name=f"avT{g}")
            nc.vector.tensor_copy(t[:], pavs.pop(g)[:])
            avT[g] = t
            for i in range(G):
                s = sigT[4 * g + i]
                nc.vector.tensor_mul(s[:], t[:], s[:])

        gg = [(nblk, m) for nblk in range(4) for m in range(4)]
        for j in range(12):
            emit_gate(*gg[j])
        emit_vrow(0)
        emit_gate(*gg[12])
        emit_scores(0, 0)
        emit_scores(0, 1)
        emit_gate(*gg[13])
        emit_softmax(0, 0)
        emit_scores(0, 2)
        emit_softmax(0, 1)
        emit_transposes(0, 0)
        emit_gate(*gg[14])
        emit_scores(0, 3)
        emit_softmax(0, 2)
        emit_transposes(0, 1)
        emit_gate(*gg[15])
        emit_vrow(1)
        emit_softmax(0, 3)
        emit_transposes(0, 2)
        emit_transposes(0, 3)
        emit_av_mm(0)
        emit_vtc_load(2)
        emit_scores(1, 0)
        emit_scores(1, 1)
        emit_av_drain(0)
        for g in range(1, KV):
            emit_softmax(g, 0)
            emit_scores(g, 2)
            emit_softmax(g, 1)
            emit_transposes(g, 0)
            emit_scores(g, 3)
            emit_softmax(g, 2)
            emit_transposes(g, 1)
            emit_softmax(g, 3)
            emit_transposes(g, 2)
            emit_transposes(g, 3)
            emit_av_mm(g)
            if g + 2 < KV:
                emit_vtc_load(g + 2)
            if g + 1 < KV:
                emit_vrow(g + 1)
                emit_scores(g + 1, 0)
                emit_scores(g + 1, 1)
            emit_av_drain(g)

        gat = sigT   # gating applied in-place per-g inside the pipeline

        # ---- out projection (bf16, host-staged wide slabs, ring reuse) ----
        wob = []
        for cc in range(KT):
            t = sb.tile([P, HS], bf16, tag="wslab", bufs=16, name=f"wo{cc}")
            nc.sync.dma_start(t[:], wo_d[cc * P:(cc + 1) * P, :])
            wob.append(t)
        for nblk in range(4):
            for rt in range(SLOTS):
                po = ps.tile([P, CHUNK], f32, tag="ps")
                for cc in range(KT):
                    nc.tensor.matmul(
                        po[:], gat[cc][:, rt * P:(rt + 1) * P],
                        wob[cc][:, nblk * CHUNK:(nblk + 1) * CHUNK],
                        start=(cc == 0), stop=(cc == KT - 1))
                t = sb.tile([P, CHUNK], f32, tag="oev", bufs=2)
                nc.scalar.copy(t[:], po[:])
                nc.sync.dma_start(
                    out_d[rt * P:(rt + 1) * P, nblk * CHUNK:(nblk + 1) * CHUNK], t[:])

    nc.compile()
    return nc


def kernel(hidden_states, cos, sin, attention_mask, Wq, Wk, Wv, Wo):
    from concourse.bass_utils import run_bass_kernel_spmd

    hidden_states = np.asarray(hidden_states, dtype=np.float32)
    cos = np.asarray(cos, dtype=np.float32)
    sin = np.asarray(sin, dtype=np.float32)
    mask = np.asarray(attention_mask, dtype=np.float32)[0, 0]
    Wq = np.asarray(Wq, dtype=np.float32)
    Wk = np.asarray(Wk, dtype=np.float32)
    Wv = np.asarray(Wv, dtype=np.float32)
    Wo = np.asarray(Wo, dtype=np.float32)

    classes = _mask_classes(mask)
    uniq_idx, n_uniq = _dedup_map(mask, classes)
    key = (tuple(tuple(r) for r in classes), tuple(uniq_idx), n_uniq)
    if key not in _CACHE:
        _CACHE[key] = _build(classes, uniq_idx, n_uniq)
    nc = _CACHE[key]

    wqg = np.ascontiguousarray(Wq[:, HS:]).astype(ml_dtypes.bfloat16)
    wob = Wo.astype(ml_dtypes.bfloat16)

    in_maps = []
    for core in range(NCORES):
        b, j = divmod(core, RANKS)
        blocks = [RANKS * k + j for k in range(SLOTS)]
        rows = np.concatenate([np.arange(bi * P, (bi + 1) * P) for bi in blocks])
        strips = _mask_strips(mask, classes, j)
        if n_uniq > 0 and len(set(uniq_idx)) != len(strips):
            # staged per unique index
            uniq_strips = [None] * (max(uniq_idx) + 1)
            for si, ui in enumerate(uniq_idx):
                if uniq_strips[ui] is None:
                    uniq_strips[ui] = strips[si]
            strips = uniq_strips
        if not strips:
            strips = [np.zeros((P, CHUNK), np.float32)]
        hidT = np.ascontiguousarray(hidden_states[b][rows].T)
        in_maps.append({
            "hidT": hidT,
            "wqg": wqg,
            "wk": Wk,
            "wv": Wv,
            "wo": wob,
            "cosT": np.ascontiguousarray(cos[b][rows].T),
            "sinT": np.ascontiguousarray(sin[b][rows].T),
            "maskst": np.ascontiguousarray(np.stack(strips)),
        })

    res = run_bass_kernel_spmd(nc, in_maps, core_ids=list(range(NCORES)))

    out = np.empty((B, S, HS), np.float32)
    for core in range(NCORES):
        b, j = divmod(core, RANKS)
        o = res.results[core]["out"]
        for k in range(SLOTS):
            bi = RANKS * k + j
            out[b, bi * P:(bi + 1) * P, :] = o[k * P:(k + 1) * P, :]
    return out


# revision 29
# speedup vs baseline: 1.1038x; 1.0225x over previous
"""Trainium2 Bass kernel for nn_Attention_34351148434119 (8 NeuronCores).

Reference computation (faithful quirks included):
  q_proj = hid @ Wq; q, gate = split(q_proj)     # q is DEAD code downstream
  k = hid @ Wk; v = hid @ Wv                     # [B,KV,S,D]
  v = RoPE(v)  (k is NOT roped; q roped but unused)
  scores = (k @ v^T) * sqrt(D) + mask; attn = softmax_t(scores)   # per kv head
  out = (tile_G(attn @ v) * sigmoid(gate)) @ Wo

Sharding: core = b*4 + j  (b = batch, j = rank in 4-core batch group).
Per batch, S=2048 is split into 16 blocks of 128 rows; core j owns blocks
{j, 4+j, 8+j, 12+j} (slot k block = 4k+j) so every core has an identical
causal workload (uniform SPMD graph; per-core specialization only via
staged data).

Pipeline (v1 lessons: collectives starve host DMA queues while in
flight, and the PE clock ramps with uninterrupted streak length):
  1. v-projection only (fp32r: hw-internal bf16 hi/lo split at full PE
     speed), RoPE, stage, AllGather issued EARLY (~t=45us).
  2. k-projection (fp32r) from the resident hid tiles.
  3. Gate matmuls run bf16 from HOST-staged bf16 weights, fully
     preloaded into SBUF before the AG starts - zero DMA during the AG.
  4. Attention per kv head: fp32r scores (logits sigma~105: softmax is
     near-argmax, bf16 anywhere in k/v->scores flips rows), two-phase
     softmax, PE transposes, bf16 attn@v.  Row-major v is derived from
     the gathered d-major v by on-chip transposes (no 2nd AllGather).
  5. Gating + bf16 out-projection from host-staged bf16 Wo, streamed
     through a deep slab ring (no conversion ops).
"""
import sys
import numpy as np
import ml_dtypes

sys.path.insert(0, "/opt/trn_rl_repo")

B, S, HS = 2, 2048, 2048
H, KV, D = 16, 4, 128
G = H // KV
SCALING = float(D) ** 0.5
P = 128
NB = S // P            # 16 row blocks per batch
NCORES = 8
RANKS = 4              # cores per batch group
SLOTS = 4              # owned 128-row blocks per core
ROWS = SLOTS * P       # 512 rows per core
CHUNK = 512            # t-chunk = 4 t-tiles
NCHUNK = S // CHUNK    # 4
KT = HS // P           # 16 contraction tiles
KVD = KV * D
NEG_THRESH = -1e8

_CACHE = {}


def _mask_classes(mask):
    """Classify each (s-slot k, t-chunk c) 512x512 region of the SxS mask.

    0 = skip (everything <= NEG_THRESH: contributes exact 0 after softmax)
    1 = plain (all zeros: no add needed)
    2 = add  (mixed: stage values and add on-chip)
    Slot k rows across all cores = blocks 4k..4k+3 = rows [512k, 512k+512).
    """
    cls = [[0] * NCHUNK for _ in range(SLOTS)]
    for k in range(SLOTS):
        for c in range(NCHUNK):
            reg = mask[512 * k:512 * (k + 1), 512 * c:512 * (c + 1)]
            if (reg <= NEG_THRESH).all():
                cls[k][c] = 0
            elif (reg == 0).all():
                cls[k][c] = 1
            else:
                cls[k][c] = 2
    ok = True
    for k in range(SLOTS):
        comp = [c for c in range(NCHUNK) if cls[k][c] != 0]
        # computed chunks must be a prefix starting at 0
        if comp != list(range(len(comp))) or 0 not in comp:
            ok = False
    if ok:
        # {k : chunk c computed} must be a suffix of slots for each c
        for c in range(NCHUNK):
            ks = [k for k in range(SLOTS) if cls[k][c] != 0]
            if ks != list(range(SLOTS - len(ks), SLOTS)):
                ok = False
    if not ok:
        # fully dense fallback: always correct for any mask
        cls = [[2] * NCHUNK for _ in range(SLOTS)]
    return cls


def _mask_strips(mask, classes, j):
    """Per-core class-2 strips, in (k, c) scan order."""
    strips = []
    for k in range(SLOTS):
        for c in range(NCHUNK):
            if classes[k][c] == 2:
                bi = RANKS * k + j
                strips.append(np.ascontiguousarray(
                    mask[bi * P:(bi + 1) * P, c * CHUNK:(c + 1) * CHUNK]))
    return strips


def _dedup_map(mask, classes):
    """Map each class-2 (k,c) to a unique-strip index, valid for EVERY
    core (cores hold different rows, so strip equality must hold on all
    of them).  Returns (uniq_of_addidx, n_uniq) or None if coreswise
    inconsistent."""
    n_add = sum(1 for k in range(SLOTS) for c in range(NCHUNK)
                if classes[k][c] == 2)
    per_core = []
    for j in range(RANKS):
        strips = _mask_strips(mask, classes, j)
        uniq = []
        idx = []
        for s in strips:
            for ui, u in enumerate(uniq):
                if np.array_equal(s, u):
                    idx.append(ui)
                    break
            else:
                uniq.append(s)
                idx.append(len(uniq) - 1)
        per_core.append(tuple(idx))
    if len(set(per_core)) != 1:
        return tuple(range(n_add)), n_add     # no dedup
    return per_core[0], max(per_core[0]) + 1 if per_core[0] else 0


def _build(classes, uniq_idx, n_uniq):
    from contextlib import ExitStack

    from concourse import bacc, mybir, tile
    from concourse.masks import make_identity

    f32 = mybir.dt.float32
    f32r = mybir.dt.float32r
    bf16 = mybir.dt.bfloat16
    Alu = mybir.AluOpType
    Act = mybir.ActivationFunctionType

    computed = [[c for c in range(NCHUNK) if classes[k][c] != 0] for k in range(SLOTS)]
    add_idx = {}
    for k in range(SLOTS):
        for c in range(NCHUNK):
            if classes[k][c] == 2:
                add_idx[(k, c)] = len(add_idx)
    n_mask = max(n_uniq, 1)
    resident_mask = n_uniq <= 4

    nc = bacc.Bacc("TRN2", target_bir_lowering=False, debug=False,
                   num_devices=NCORES)

    hidT_d = nc.declare_dram_parameter("hidT", [HS, ROWS], f32r, isOutput=False)
    wqg_d = nc.declare_dram_parameter("wqg", [HS, HS], bf16, isOutput=False)
    wkv_d = nc.declare_dram_parameter("wkv", [HS, 2 * KVD], f32r, isOutput=False)
    wo_d = nc.declare_dram_parameter("wo", [HS, HS], bf16, isOutput=False)
    cosT_d = nc.declare_dram_parameter("cosT", [D, ROWS], f32, isOutput=False)
    sinT_d = nc.declare_dram_parameter("sinT", [D, ROWS], f32, isOutput=False)
    mask_d = nc.declare_dram_parameter("maskst", [n_mask, P, CHUNK], f32,
                                       isOutput=False)
    out_d = nc.declare_dram_parameter("out", [ROWS, HS], f32, isOutput=True)

    rg = [[0, 1, 2, 3], [4, 5, 6, 7]]
    NSLAB = 64    # bf16 weight-slab ring (full gate preload, wo streams through)

    with tile.TileContext(nc) as tc, ExitStack() as ctx:
        sb = ctx.enter_context(tc.tile_pool(name="sb", bufs=2))
        ps = ctx.enter_context(tc.tile_pool(name="ps", bufs=8, space="PSUM"))
        dram = ctx.enter_context(tc.tile_pool(name="dram", bufs=1, space="DRAM"))

        # ---- constants ----
        id_f32 = sb.tile([P, P], f32, tag="c_idf")
        id_bf = sb.tile([P, P], bf16, tag="c_idb")
        make_identity(nc, id_f32[:])
        make_identity(nc, id_bf[:])

        # ---- combined k+v projection (fp32r): the front half is at the
        # aggregate HBM roofline, so k matmuls ride the same DMA-bound
        # window as v instead of consuming PE time afterwards ----
        pv = [ps.tile([P, ROWS], f32, tag="ps", name=f"pv{g}") for g in range(KV)]
        pk = [ps.tile([P, ROWS], f32, tag="ps", name=f"pk{g}") for g in range(KV)]
        hidb = []
        hfw = [sb.tile([P, 4 * ROWS], f32r, tag="f32big", bufs=2, name=f"hfw{q}")
               for q in range(4)]
        hid = [hfw[kk // 4][:, (kk % 4) * ROWS:(kk % 4 + 1) * ROWS]
               for kk in range(KT)]
        for kk in range(KT):
            nc.sync.dma_start(hid[kk], hidT_d[kk * P:(kk + 1) * P, :])
            wt = sb.tile([P, 2 * KVD], f32r, tag="wkv", bufs=2, name=f"wv{kk}")
            nc.gpsimd.dma_start(wt[:], wkv_d[kk * P:(kk + 1) * P, :])
            for g in range(KV):
                nc.tensor.matmul(pv[g][:], wt[:, g * P:(g + 1) * P], hid[kk],
                                 start=(kk == 0), stop=(kk == KT - 1))
            for g in range(KV):
                nc.tensor.matmul(pk[g][:], wt[:, KVD + g * P:KVD + (g + 1) * P],
                                 hid[kk], start=(kk == 0), stop=(kk == KT - 1))
            hb = sb.tile([P, ROWS], bf16, tag="hidb", bufs=16, name=f"hb{kk}")
            nc.scalar.copy(hb[:], hid[kk].bitcast(f32))
            hidb.append(hb)

        # ---- cos/sin/mask after the kv stream (needed only ~t=75) ----
        cosT = sb.tile([D, ROWS], f32, tag="c_cos")
        sinT = sb.tile([D, ROWS], f32, tag="c_sin")
        nc.sync.dma_start(cosT[:], cosT_d[:, :])
        nc.sync.dma_start(sinT[:], sinT_d[:, :])
        mtiles = []
        if resident_mask:
            for u in range(n_uniq):
                mt = sb.tile([P, CHUNK], f32, tag="msk", bufs=max(n_uniq, 1),
                             name=f"mt{u}")
                nc.sync.dma_start(mt[:], mask_d[u, :, :])
                mtiles.append(mt)

        # ---- gate weights: wide slabs on the SCALAR dma queue ----
        wqb = []
        for kk in range(KT):
            t = sb.tile([P, HS], bf16, tag="wslab", bufs=16, name=f"wq{kk}")
            nc.scalar.dma_start(t[:], wqg_d[kk * P:(kk + 1) * P, :])
            wqb.append(t)

        # ---- RoPE v pre-AG; per-g AllGather (4 small collectives) so
        # head 0's data lands well before attention needs it ----
        vt_in = dram.tile([KVD, ROWS], f32)
        vt_all = [dram.tile([RANKS * P, ROWS], f32, name=f"vtall{g}")
                  for g in range(KV)]
        for g in range(KV):
            vr = sb.tile([P, ROWS], f32, tag="vraw", bufs=2, name=f"vr{g}")
            nc.scalar.copy(vr[:], pv[g][:])
            rot = sb.tile([P, ROWS], f32, tag="vrot", bufs=2, name=f"rot{g}")
            nc.vector.tensor_scalar_mul(rot[0:64, :], vr[64:128, :], -1.0)
            nc.vector.tensor_copy(rot[64:128, :], vr[0:64, :])
            nc.vector.tensor_mul(vr[:], vr[:], cosT[:])
            nc.vector.tensor_mul(rot[:], rot[:], sinT[:])
            nc.vector.tensor_add(vr[:], vr[:], rot[:])
            nc.gpsimd.dma_start(vt_in[g * P:(g + 1) * P, :], vr[:])
            nc.gpsimd.collective_compute(
                "AllGather", mybir.AluOpType.bypass, replica_groups=rg,
                ins=[vt_in[g * P:(g + 1) * P, :].opt()],
                outs=[vt_all[g].opt()])

        kT = []   # per g: [128 d, 512 rows] f32r, pre-scaled by sqrt(D)
        for g in range(KV):
            t = sb.tile([P, ROWS], f32r, tag="kT", bufs=4, name=f"kT{g}")
            nc.scalar.mul(t[:], pk[g][:], SCALING)
            kT.append(t)

        # ---- gathered v: per-g wide tiles recycling the hid ring
        # (bufs=2).  g=0,1 load right after the AG; g=2,3 are deferred
        # into the attention pipeline so their ring waits never block
        # the queue mid-phase. ----
        vtcg = []

        def emit_vtc_load(g):
            blocks = vtcg[g][:].rearrange("p (b c) -> p b c", b=NB)
            for r in range(RANKS):
                nc.sync.dma_start(
                    blocks[:, r::RANKS, :],
                    vt_all[g][r * P:(r + 1) * P, :].bitcast(f32r)
                    .rearrange("p (b c) -> p b c", b=NCHUNK))

        for g in range(KV):
            vtcg.append(sb.tile([P, 4 * CHUNK], f32r, tag="f32big", bufs=2,
                                name=f"vtcg{g}"))
        emit_vtc_load(0)
        emit_vtc_load(1)

        # ---- gate matmul (bf16) + fused sigmoid: zero DMA during AG;
        # the last 6 groups interleave with head-0 attention so the PE
        # covers the softmax chain ----
        sigT = [None] * 16

        def emit_gate(nblk, m):
            pg = ps.tile([P, ROWS], f32, tag="ps", name=f"pg{nblk}_{m}")
            for kk in range(KT):
                nc.tensor.matmul(
                    pg[:], wqb[kk][:, nblk * CHUNK + m * P:nblk * CHUNK + (m + 1) * P],
                    hidb[kk][:], start=(kk == 0), stop=(kk == KT - 1))
            t = sb.tile([P, ROWS], bf16, tag="sigT", bufs=16, name=f"sig{nblk}_{m}")
            nc.scalar.activation(t[:], pg[:], Act.Sigmoid)
            sigT[nblk * 4 + m] = t

        # ---- attention: flat software pipeline over (g, slot) tasks.
        # scores of slot k+1/k+2 and transposes of slot k-1 overlap the
        # Act/DVE softmax chain of slot k, so the PE never drains. ----
        avT = [None] * KV
        vrows = {}
        attnTs = {}
        sstate = {}
        tstate = {}
        pavs = {}
        def emit_vrow(g):
            vl = []
            for bi in range(NB):
                tp = ps.tile([P, P], f32, tag="ps", name=f"tvp{g}_{bi}")
                nc.tensor.transpose(
                    tp[:], vtcg[g][:, bi * P:(bi + 1) * P].bitcast(f32), id_f32[:])
                t = sb.tile([P, P], bf16, tag="vrow", bufs=32,
                            name=f"vrow{g}_{bi}")
                if bi % 2:
                    nc.scalar.copy(t[:], tp[:])
                else:
                    nc.vector.tensor_copy(t[:], tp[:])
                vl.append(t)
            vrows[g] = vl
            attnTs[g] = [sb.tile([P, ROWS], bf16, tag="attnT", bufs=16,
                                 name=f"attnT{g}_{bi}") for bi in range(NB)]

        def emit_scores(g, k):
            comp = computed[k]
            pscs = []
            cms = []
            for ci, c in enumerate(comp):
                psc = ps.tile([P, CHUNK], f32, tag="ps", name=f"psc{g}_{k}_{ci}")
                nc.tensor.matmul(psc[:], kT[g][:, k * P:(k + 1) * P],
                                 vtcg[g][:, c * CHUNK:(c + 1) * CHUNK],
                                 start=True, stop=True)
                if classes[k][c] == 2:
                    ai = add_idx[(k, c)]
                    if resident_mask:
                        mt = mtiles[uniq_idx[ai]]
                    else:
                        mt = sb.tile([P, CHUNK], f32, tag="msk", bufs=4,
                                     name=f"msk{g}_{k}_{c}")
                        nc.gpsimd.dma_start(mt[:], mask_d[ai, :, :])
                    nc.vector.tensor_add(psc[:], psc[:], mt[:])
                cm = sb.tile([P, 1], f32, tag="stat", bufs=32,
                             name=f"cm{g}_{k}_{ci}")
                nc.vector.tensor_reduce(cm[:], psc[:], mybir.AxisListType.X,
                                        Alu.max, negate=True)
                pscs.append(psc)
                cms.append(cm)
            sstate[(g, k)] = (pscs, cms, comp)

        def emit_softmax(g, k):
            pscs, cms, comp = sstate.pop((g, k))
            nchk = len(comp)
            attn = sb.tile([P, CHUNK * nchk], bf16, tag="attn", bufs=3,
                           padded_shape=[P, CHUNK * NCHUNK],
                           name=f"attn{g}_{k}")
            mneg = cms[0]   # -max
            for ci in range(1, nchk):
                mnew = sb.tile([P, 1], f32, tag="stat", bufs=32,
                               name=f"mn{g}_{k}_{ci}")
                nc.vector.tensor_tensor(mnew[:], mneg[:], cms[ci][:], Alu.min)
                mneg = mnew
            tot = None
            for ci in range(nchk):
                csum = sb.tile([P, 1], f32, tag="stat", bufs=32,
                               name=f"cs{g}_{k}_{ci}")
                nc.scalar.activation(attn[:, ci * CHUNK:(ci + 1) * CHUNK],
                                     pscs[ci][:], Act.Exp, bias=mneg[:],
                                     accum_out=csum[:])
                if tot is None:
                    tot = csum
                else:
                    t2 = sb.tile([P, 1], f32, tag="stat", bufs=32,
                                 name=f"tt{g}_{k}_{ci}")
                    nc.vector.tensor_add(t2[:], tot[:], csum[:])
                    tot = t2
            rinv = sb.tile([P, 1], f32, tag="stat", bufs=32,
                           name=f"ri{g}_{k}")
            nc.vector.reciprocal(rinv[:], tot[:])
            for ci in range(nchk):
                nc.vector.tensor_scalar_mul(
                    attn[:, ci * CHUNK:(ci + 1) * CHUNK],
                    attn[:, ci * CHUNK:(ci + 1) * CHUNK], rinv[:])
            tstate[(g, k)] = (attn, comp)

        def emit_transposes(g, k):
            attn, comp = tstate.pop((g, k))
            attnT = attnTs[g]
            for ci, c in enumerate(comp):
                for i in range(4):
                    bi = 4 * c + i
                    tp = ps.tile([P, P], bf16, tag="ps", name=f"tap{g}_{k}_{bi}")
                    nc.tensor.transpose(
                        tp[:], attn[:, ci * CHUNK + i * P:ci * CHUNK + (i + 1) * P],
                        id_bf[:])
                    if i % 2:
                        nc.scalar.copy(attnT[bi][:, k * P:(k + 1) * P], tp[:])
                    else:
                        nc.vector.tensor_copy(attnT[bi][:, k * P:(k + 1) * P], tp[:])

        def emit_av_mm(g):
            pav = ps.tile([P, ROWS], f32, tag="ps", name=f"pav{g}")
            first = True
            for bi in range(NB):
                ks = [k for k in range(SLOTS) if (bi // RANKS) in computed[k]]
                if not ks:
                    continue
                kmin = ks[0]
                nc.tensor.matmul(pav[:, kmin * P:ROWS], vrows[g][bi][:],
                                 attnTs[g][bi][:, kmin * P:ROWS],
                                 start=first, stop=(bi == NB - 1))
                first = False
            pavs[g] = pav

        def emit_av_drain(g):
            t = sb.tile([P, ROWS], bf16, tag="avT", bufs=4, name=f"avT{g}")
            nc.vector.tensor_copy(t[:], pavs.pop(g)[:])
            avT[g] = t
            for i in range(G):
                s = sigT[4 * g + i]
                nc.vector.tensor_mul(s[:], t[:], s[:])

        gg = [(nblk, m) for nblk in range(4) for m in range(4)]
        for j in range(12):
            emit_gate(*gg[j])
        emit_vrow(0)
        emit_gate(*gg[12])
        emit_scores(0, 0)
        emit_scores(0, 1)
        emit_gate(*gg[13])
        emit_softmax(0, 0)
        emit_scores(0, 2)
        emit_softmax(0, 1)
        emit_transposes(0, 0)
        emit_gate(*gg[14])
        emit_scores(0, 3)
        emit_softmax(0, 2)
        emit_transposes(0, 1)
        emit_gate(*gg[15])
        emit_vrow(1)
        emit_softmax(0, 3)
        emit_transposes(0, 2)
        emit_transposes(0, 3)
        emit_av_mm(0)
        emit_vtc_load(2)
        emit_scores(1, 0)
        emit_scores(1, 1)
        emit_av_drain(0)
        for g in range(1, KV):
            emit_softmax(g, 0)
            emit_scores(g, 2)
            emit_softmax(g, 1)
            emit_transposes(g, 0)
            emit_scores(g, 3)
            emit_softmax(g, 2)
            emit_transposes(g, 1)
            emit_softmax(g, 3)
            emit_transposes(g, 2)
            emit_transposes(g, 3)
            emit_av_mm(g)
            if g + 2 < KV:
                emit_vtc_load(g + 2)
            if g + 1 < KV:
                emit_vrow(g + 1)
                emit_scores(g + 1, 0)
                emit_scores(g + 1, 1)
            emit_av_drain(g)

        gat = sigT   # gating applied in-place per-g inside the pipeline

        # ---- out projection (bf16, host-staged wide slabs, ring reuse) ----
        wob = []
        for cc in range(KT):
            t = sb.tile([P, HS], bf16, tag="wslab", bufs=16, name=f"wo{cc}")
            nc.sync.dma_start(t[:], wo_d[cc * P:(cc + 1) * P, :])
            wob.append(t)
        for nblk in range(4):
            for rt in range(SLOTS):
                po = ps.tile([P, CHUNK], f32, tag="ps")
                for cc in range(KT):
                    nc.tensor.matmul(
                        po[:], gat[cc][:, rt * P:(rt + 1) * P],
                        wob[cc][:, nblk * CHUNK:(nblk + 1) * CHUNK],
                        start=(cc == 0), stop=(cc == KT - 1))
                t = sb.tile([P, CHUNK], f32, tag="oev", bufs=2)
                nc.scalar.copy(t[:], po[:])
                nc.sync.dma_start(
                    out_d[rt * P:(rt + 1) * P, nblk * CHUNK:(nblk + 1) * CHUNK], t[:])

    nc.compile()
    return nc


def kernel(hidden_states, cos, sin, attention_mask, Wq, Wk, Wv, Wo):
    from concourse.bass_utils import run_bass_kernel_spmd

    hidden_states = np.asarray(hidden_states, dtype=np.float32)
    cos = np.asarray(cos, dtype=np.float32)
    sin = np.asarray(sin, dtype=np.float32)
    mask = np.asarray(attention_mask, dtype=np.float32)[0, 0]
    Wq = np.asarray(Wq, dtype=np.float32)
    Wk = np.asarray(Wk, dtype=np.float32)
    Wv = np.asarray(Wv, dtype=np.float32)
    Wo = np.asarray(Wo, dtype=np.float32)

    classes = _mask_classes(mask)
    uniq_idx, n_uniq = _dedup_map(mask, classes)
    key = (tuple(tuple(r) for r in classes), tuple(uniq_idx), n_uniq)
    if key not in _CACHE:
        _CACHE[key] = _build(classes, uniq_idx, n_uniq)
    nc = _CACHE[key]

    wqg = np.ascontiguousarray(Wq[:, HS:]).astype(ml_dtypes.bfloat16)
    wkv = np.ascontiguousarray(np.hstack([Wv, Wk]))
    wob = Wo.astype(ml_dtypes.bfloat16)

    in_maps = []
    for core in range(NCORES):
        b, j = divmod(core, RANKS)
        blocks = [RANKS * k + j for k in range(SLOTS)]
        rows = np.concatenate([np.arange(bi * P, (bi + 1) * P) for bi in blocks])
        strips = _mask_strips(mask, classes, j)
        if n_uniq > 0 and len(set(uniq_idx)) != len(strips):
            # staged per unique index
            uniq_strips = [None] * (max(uniq_idx) + 1)
            for si, ui in enumerate(uniq_idx):
                if uniq_strips[ui] is None:
                    uniq_strips[ui] = strips[si]
            strips = uniq_strips
        if not strips:
            strips = [np.zeros((P, CHUNK), np.float32)]
        hidT = np.ascontiguousarray(hidden_states[b][rows].T)
        in_maps.append({
            "hidT": hidT,
            "wqg": wqg,
            "wkv": wkv,
            "wo": wob,
            "cosT": np.ascontiguousarray(cos[b][rows].T),
            "sinT": np.ascontiguousarray(sin[b][rows].T),
            "maskst": np.ascontiguousarray(np.stack(strips)),
        })

    res = run_bass_kernel_spmd(nc, in_maps, core_ids=list(range(NCORES)))

    out = np.empty((B, S, HS), np.float32)
    for core in range(NCORES):
        b, j = divmod(core, RANKS)
        o = res.results[core]["out"]
        for k in range(SLOTS):
            bi = RANKS * k + j
            out[b, bi * P:(bi + 1) * P, :] = o[k * P:(k + 1) * P, :]
    return out
